# revision 57
# baseline (speedup 1.0000x reference)
"""LRFGraphConv Trainium2 kernel.

Math: for each vertex i with neighbors N(i) (directed edge list, src=center):
    out[i] = ((sum_{j in N(i)} verts[j] - deg_i * verts[i]) @ lrf[i]) @ W.T + maxN * b

The neighbor-sum commutes with the per-center rotation and GEMM, so the
per-edge work collapses to a segment-sum of neighbor coordinates.  The
rotation and GEMM fuse into a single tensor-engine contraction over the 9
(j,k) pairs of u[i,(j,k)] = t[i,j]*lrf[i,j,k] against Wrep[(j,k),n] = W[n,k],
plus a constant-1 row carrying the maxN*b bias.  u uses 16 slots per vertex
(9 real + bias + 6 pad) so the GEMM runs as an h0/h64 row-group pair that
co-executes on the PE's upper/lower 64-row halves (~605ns for 1024 cols).

Sharding: vertices are partitioned contiguously across 8 cores (6250 each),
then sorted by degree (ascending) within each core.  The host buckets
directed edges by owner of src and builds per-chunk padded neighbor tables
whose slot count is that chunk's max degree + 1 (the "+1" fold slot holds
-deg*verts) -- low-degree chunks get narrow tables, so table bytes and
reduce work drop ~20% vs a uniform cap, and no overflow tier is needed.
All inputs are packed into one dram blob; each load group carries its own
lrf slice so the first multiply is never gated on a big aux transfer (a DMA
issue costs ~600ns of engine queue time and completion has a ~1.5us fixed
latency, so loads are few and early).  The W panel loads in parallel from
the GpSimd queue so it never delays the chunk-table stream on Sync, and the
first three chunks load individually so the pipeline fills at DMA latency.  Engine assignment per chunk:
  DVE:    per-chunk slot reduce + uT (transpose PSUM->SBUF) copies
  Pool:   u = t*lrf broadcast multiply; issues the final output store
  PE:     transpose + GEMM pair (fp16)
  Act:    all PSUM output drains (fp32->fp16) -- keeping DVE free for the
          reduce/uT critical path measurably beats any drain split
  Sync:   input loads + per-chunk output stores
Each chunk's output region is stored as soon as it drains; the last chunk
is 1 tile so the final store -- which gates the graded NEFF teardown
(semaphore-clear storm + barriers, ~7.7us fixed) -- is tiny.  The walrus
backend runs with --policy=3 (time-aware post-scheduler) plus DMA-ordering
flags, which both speeds the schedule and cuts run-to-run variance.
No collectives.
"""

import os
import sys

sys.path.insert(0, "/opt/trn_rl_repo")

import numpy as np

import concourse.bass as bass
import concourse.bacc as bacc
import concourse.tile as tile
from concourse import mybir
from concourse.masks import make_identity
from concourse.bass_utils import run_bass_kernel_spmd
import concourse.bass_utils as _bu

# time-aware post-scheduler: measurably faster and far lower run-to-run
# variance for this kernel than the default --policy=0
if not getattr(_bu, "_kb_policy3", False):
    _bu._kb_policy3 = True
    _orig_gwa = _bu.get_walrus_args
    def _gwa(*a, **kw):
        return _orig_gwa(*a, **kw) + ["--policy=3", "--global-dma-ordering-optimization=1", "--enable-hwdge-trigger-engine-scheduling"]
    _bu.get_walrus_args = _gwa

V = 50000
NCORES = 8
VC = V // NCORES          # 6250 owned vertices per core
P = 128
NVT = (VC + P - 1) // P   # 49 vertex tiles per core
VCP = NVT * P             # 6272 padded
MAXNV = 8                 # tiles per chunk (PSUM: 8*128 fp32 = 2 banks)

CHUNKS = [4, 6, 8, 8, 8, 8, 6, 1]
assert sum(CHUNKS) == NVT
NCH = len(CHUNKS)
# load groups: chunks loaded together (each group also carries its aux slice)
LGROUPS = [[0], [1], [2], [3, 4], [5, 6], [7]]

BF = mybir.dt.float16
BF_NP = np.float16

LAST_RESULTS = None       # BassKernelResults of the most recent run (for test.py)


def build(nc: bass.Bass, NPC):
    """NPC[c] = slot count (max degree + 1 fold slot) for chunk c."""
    dt = mybir.dt
    vstart = [0]
    for nv in CHUNKS:
        vstart.append(vstart[-1] + nv)

    # ---- packed input blob layout (fp16 cols per partition row) ----
    # group 0 tables+aux | wr | group 1 tables+aux | group 2 ... |
    W_COLS = 512
    o_xp = [0] * NCH
    o_aux = [0] * NCH
    off = 0
    group_span = []
    for gi, chs in enumerate(LGROUPS):
        glo = off
        for c in chs:
            o_xp[c] = off
            off += CHUNKS[c] * 3 * NPC[c]
        for c in chs:
            o_aux[c] = off
            off += CHUNKS[c] * 9
        group_span.append((glo, off))
        if gi == 0:
            o_wr = off
            off += W_COLS
    TOT = off

    blob = nc.dram_tensor("blobf31", [P, TOT], BF, kind="ExternalInput")
    out = nc.dram_tensor("out", [P, NVT * P], dt.float16, kind="ExternalOutput")

    with tile.TileContext(nc) as tc:
        with (
            tc.tile_pool(name="c", bufs=1) as cpool,
            tc.tile_pool(name="w", bufs=8) as wpool,
            tc.tile_pool(name="pt", bufs=2, space="PSUM") as pst,
            tc.tile_pool(name="pg", bufs=3, space="PSUM") as psg,
        ):
            outsb = cpool.tile([P, NVT * P], dt.float16)
            ident = cpool.tile([P, P], BF)
            with tc.high_priority():
                make_identity(nc, ident[:])
            xsb = cpool.tile([P, TOT], BF, tag="xsb")
            w_t = xsb[:, o_wr : o_wr + W_COLS]
            # loads: table groups stream on Sync; the W panel loads in
            # parallel from GpSimd so it never delays the chunk tables
            nc.sync.dma_start(
                out=xsb[:, group_span[0][0] : group_span[0][1]],
                in_=blob[:, group_span[0][0] : group_span[0][1]],
            )
            nc.gpsimd.dma_start(
                out=xsb[:, o_wr : o_wr + W_COLS], in_=blob[:, o_wr : o_wr + W_COLS]
            )
            for lo, hi in group_span[1:]:
                nc.sync.dma_start(out=xsb[:, lo:hi], in_=blob[:, lo:hi])

            # persistent u tiles (4-deep rotation); bias slot 9 = 1, 10:16 = 0
            u_bufs = []
            for s in range(4):
                ub = cpool.tile([P, MAXNV * 16], BF, tag=f"u{s}")
                nc.gpsimd.memset(ub[:], 0.0)
                nc.gpsimd.memset(
                    ub[:].rearrange("p (v s) -> p v s", s=16)[:, :, 9:10], 1.0
                )
                u_bufs.append(ub)

            state = [None] * NCH  # per-chunk (u, uT, pg)

            def stage_reduce_mult(c):
                nv = CHUNKS[c]
                npc = NPC[c]
                xv = xsb[:, o_xp[c] : o_xp[c] + nv * 3 * npc].rearrange(
                    "p (v c n) -> p v c n", v=nv, c=3, n=npc
                )
                t = wpool.tile([P, MAXNV * 3], BF, tag="t")
                with nc.allow_low_precision(reason="fp16 neighbor sums"):
                    nc.vector.tensor_reduce(
                        out=t[:, : nv * 3], in_=xv,
                        axis=mybir.AxisListType.X,
                        op=mybir.AluOpType.add,
                    )
                # u[p, v, k*3+j] = t[p,v,j]*lrf9[p,v,k*3+j] broadcast mul (Pool)
                u = u_bufs[c % 4]
                u9 = u[:, : nv * 16].rearrange("p (v s) -> p v s", s=16)[
                    :, :, 0:9
                ].rearrange("p v (k j) -> p v k j", k=3, j=3)
                aux9 = xsb[:, o_aux[c] : o_aux[c] + nv * 9].rearrange(
                    "p (v f) -> p v f", f=9
                )
                t4 = t[:, : nv * 3].rearrange("p (v c) -> p v c", c=3).unsqueeze(2)
                nc.gpsimd.tensor_tensor(
                    out=u9,
                    in0=t4.to_broadcast([P, nv, 3, 3]),
                    in1=aux9.rearrange("p v (k j) -> p v k j", k=3, j=3),
                    op=mybir.AluOpType.mult,
                )
                state[c] = [u, None, None]

            def stage_tu(c):
                nv = CHUNKS[c]
                cw = nv * 16
                u = state[c][0]
                pt = pst.tile([P, P], BF, tag="pt")
                nc.tensor.transpose(
                    out=pt[:cw, :], in_=u[:, :cw], identity=ident[:]
                )
                uT = wpool.tile([P, P], BF, tag="uT")
                nc.vector.tensor_copy(out=uT[:cw, :], in_=pt[:cw, :])
                state[c][1] = uT

            def stage_gemm(c):
                nv = CHUNKS[c]
                uT = state[c][1]
                pg = psg.tile([P, MAXNV * P], dt.float32, tag="pg")
                g = 0
                while g < nv:
                    ng = min(4, nv - g)
                    rb = 16 * g
                    nc.tensor.matmul(
                        out=pg[:, g * P : (g + ng) * P],
                        lhsT=uT[rb : rb + 16 * ng, :],
                        rhs=w_t[rb : rb + 16 * ng, : ng * P],
                        start=True,
                        stop=True,
                    )
                    g += ng
                state[c][2] = pg

            # every chunk flushes its own region right after its drain
            flush_of = {c: (c, c + 1) for c in range(NCH)}

            def stage_drain_store(c):
                nv = CHUNKS[c]
                ow = nv * P
                olo = vstart[c] * P
                pg = state[c][2]
                nc.scalar.copy(out=outsb[:, olo : olo + ow], in_=pg[:, :ow])
                if c in flush_of:
                    ca, cb = flush_of[c]
                    lo = vstart[ca] * P
                    hi = vstart[cb] * P
                    eng = nc.sync if c < NCH - 1 else nc.gpsimd
                    eng.dma_start(out=out[:, lo:hi], in_=outsb[:, lo:hi])

            # software-pipelined emission: reduce/mult 2 chunks ahead of the
            # GEMM, transpose/uT 1 ahead, drain right after its GEMM
            for i in range(NCH + 3):
                if i < NCH:
                    stage_reduce_mult(i)
                if 0 <= i - 1 < NCH:
                    stage_tu(i - 1)
                if 0 <= i - 2 < NCH:
                    stage_gemm(i - 2)
                if 0 <= i - 3 < NCH:
                    stage_drain_store(i - 3)
    return nc


def _host_prep(verts, edges, lrf, W, b):
    vb = np.asarray(verts, dtype=np.float32)
    e = np.asarray(edges).astype(np.int64)
    src = np.concatenate([e[:, 0], e[:, 1]]).astype(np.int64)
    dst = np.concatenate([e[:, 1], e[:, 0]]).astype(np.int64)

    deg = np.bincount(src, minlength=V).astype(np.int64)
    maxN = int(deg.max())

    # per-core remap: sort by degree ascending -> low-degree chunks get
    # narrow neighbor tables
    degc = deg.reshape(NCORES, VC)
    newpos = np.empty((NCORES, VC), np.int64)
    order_c = np.empty((NCORES, VC), np.int64)
    for cc in range(NCORES):
        oc = np.argsort(degc[cc], kind="stable")
        order_c[cc] = oc
        newpos[cc, oc] = np.arange(VC)

    vstart = [0]
    for nv in CHUNKS:
        vstart.append(vstart[-1] + nv)
    deg_sorted = np.sort(degc, axis=1)          # per core, ascending
    NPC = []
    for c in range(NCH):
        hi = min(vstart[c + 1] * P, VC)
        cap = int(deg_sorted[:, :hi].max())     # max over cores for SPMD
        NPC.append(cap + 1)

    order = np.argsort(src, kind="stable")
    src_s = src[order]
    dst_s = dst[order]
    starts = np.zeros(V + 1, np.int64)
    np.cumsum(deg, out=starts[1:])
    slot = np.arange(src_s.size, dtype=np.int64) - starts[src_s]

    c_a = src_s // VC
    il_new = newpos[c_a, src_s - c_a * VC]
    p_a = il_new % P
    v_a = il_new // P
    vals = vb[dst_s].astype(BF_NP)

    chunk_of_tile = np.zeros(NVT, np.int64)
    for c in range(NCH):
        chunk_of_tile[vstart[c] : vstart[c + 1]] = c
    ch_a = chunk_of_tile[v_a]

    Xp = [np.zeros((NCORES, P, CHUNKS[c], 3, NPC[c]), BF_NP) for c in range(NCH)]
    for c in range(NCH):
        m = ch_a == c
        Xp[c][c_a[m], p_a[m], v_a[m] - vstart[c], :, slot[m]] = vals[m]

    # fold slot: -deg*verts for the owned vertex goes in the last slot
    dv = (-deg[:, None].astype(np.float32)) * vb
    dv_pad = np.zeros((NCORES, VCP, 3), np.float32)
    for cc in range(NCORES):
        dv_pad[cc, :VC] = dv.reshape(NCORES, VC, 3)[cc][order_c[cc]]
    dv_t = dv_pad.reshape(NCORES, NVT, P, 3).transpose(0, 2, 1, 3)  # [NC,P,NVT,3]
    for c in range(NCH):
        Xp[c][:, :, :, :, NPC[c] - 1] = dv_t[
            :, :, vstart[c] : vstart[c + 1], :
        ].astype(BF_NP)

    # aux per vertex: lrf(9), remapped -> [NC, P, NVT, 9]
    aux_flat = np.zeros((NCORES, VCP, 9), np.float32)
    # k-major flattening: slot s = k*3+j holds lrf[:, j, k]
    lrf9 = np.ascontiguousarray(
        np.asarray(lrf, np.float32).reshape(NCORES, VC, 3, 3).transpose(0, 1, 3, 2)
    ).reshape(NCORES, VC, 9)
    for cc in range(NCORES):
        aux_flat[cc, :VC] = lrf9[cc][order_c[cc]]
    auxh = aux_flat.reshape(NCORES, NVT, P, 9).transpose(0, 2, 1, 3).astype(BF_NP)

    Wf = np.asarray(W, np.float32)
    W16 = np.zeros((16, P), np.float32)
    for s in range(9):
        W16[s, :] = Wf[:, s // 3]   # k-major: slot s = k*3+j -> k = s//3
    W16[9, :] = maxN * np.asarray(b, np.float32)
    half = np.zeros((64, 512), np.float32)
    for q in range(4):
        half[16 * q : 16 * q + 16, 128 * q : 128 * q + 128] = W16
    Wr = np.ascontiguousarray(np.vstack([half, half])).astype(BF_NP)

    in_maps = []
    for cc in range(NCORES):
        parts = []
        for gi, chs in enumerate(LGROUPS):
            for c in chs:
                parts.append(np.ascontiguousarray(Xp[c][cc].reshape(P, -1)))
            for c in chs:
                parts.append(
                    np.ascontiguousarray(
                        auxh[cc, :, vstart[c] : vstart[c + 1]].reshape(P, -1)
                    )
                )
            if gi == 0:
                parts.append(Wr)
        in_maps.append({"blobf31": np.ascontiguousarray(np.concatenate(parts, axis=1))})
    return in_maps, NPC, order_c


def kernel(verts, edges, lrf, W, b):
    global LAST_RESULTS
    in_maps, NPC, order_c = _host_prep(verts, edges, lrf, W, b)

    nc = bacc.Bacc()
    build(nc, NPC)
    nc.finalize()

    trace = os.environ.get("KBENCH_TRACE") == "1"
    res = run_bass_kernel_spmd(
        nc, in_maps, core_ids=list(range(NCORES)), trace=trace
    )
    LAST_RESULTS = res

    full = np.empty((V, 128), np.float32)
    for c in range(NCORES):
        o = (
            res.results[c]["out"].astype(np.float32)
            .reshape(P, NVT, P).transpose(1, 0, 2).reshape(VCP, P)[:VC]
        )
        blk = full[c * VC : (c + 1) * VC]
        blk[order_c[c]] = o
    return full


# revision 71
# speedup vs baseline: 1.0926x; 1.0926x over previous
"""LRFGraphConv Trainium2 kernel.

Math: for each vertex i with neighbors N(i) (directed edge list, src=center):
    out[i] = ((sum_{j in N(i)} verts[j] - deg_i * verts[i]) @ lrf[i]) @ W.T + maxN * b

The neighbor-sum commutes with the per-center rotation and GEMM, so the
per-edge work collapses to a segment-sum of neighbor coordinates.  The
rotation and GEMM fuse into a single tensor-engine contraction over the 9
(j,k) pairs of u[i,(j,k)] = t[i,j]*lrf[i,j,k] against Wrep[(j,k),n] = W[n,k],
plus a constant-1 row carrying the maxN*b bias.  u uses 16 slots per vertex
(9 real + bias + 6 pad) so the GEMM runs as an h0/h64 row-group pair that
co-executes on the PE's upper/lower 64-row halves (~605ns for 1024 cols).

Sharding: vertices are partitioned contiguously across 8 cores (6250 each),
then sorted by degree (ascending) within each core.  The host buckets
directed edges by owner of src and builds per-chunk padded neighbor tables
whose slot count is that chunk's max degree + 1 (the "+1" fold slot holds
-deg*verts) -- low-degree chunks get narrow tables, so table bytes and
reduce work drop ~20% vs a uniform cap, and no overflow tier is needed.
All inputs are packed into one dram blob; each load group carries its own
lrf slice so the first multiply is never gated on a big aux transfer (a DMA
issue costs ~600ns of engine queue time and completion has a ~1.5us fixed
latency, so loads are few and early).  The W panel loads in parallel from
the GpSimd queue so it never delays the chunk-table stream on Sync, and the
first three chunks load individually so the pipeline fills at DMA latency.  Engine assignment per chunk:
  DVE:    per-chunk slot reduce + uT (transpose PSUM->SBUF) copies
  Pool:   u = t*lrf broadcast multiply; issues the final output store
  PE:     transpose + GEMM pair (fp16)
  Act:    all PSUM output drains (fp32->fp16) -- keeping DVE free for the
          reduce/uT critical path measurably beats any drain split
  Sync:   input loads + per-chunk output stores
Each chunk's output region is stored as soon as it drains; the last chunk
is 1 tile so the final store -- which gates the graded NEFF teardown
(semaphore-clear storm + barriers, ~7.7us fixed) -- is tiny.  The walrus
backend runs with --policy=2 (heuristics post-scheduler) plus DMA-ordering
flags, which both speeds the schedule and cuts run-to-run variance.
No collectives.
"""

import os
import sys

sys.path.insert(0, "/opt/trn_rl_repo")

import numpy as np

import concourse.bass as bass
import concourse.bacc as bacc
import concourse.tile as tile
from concourse import mybir
from concourse.masks import make_identity
from concourse.bass_utils import run_bass_kernel_spmd
import concourse.bass_utils as _bu

# heuristics post-scheduler: measurably faster and far lower run-to-run
# variance for this kernel than the default --policy=0 (policy=2 beats
# policy=3 with the 8-deep t/uT buffer rotation; confirmed same-window)
if not getattr(_bu, "_kb_policy3", False):
    _bu._kb_policy3 = True
    _orig_gwa = _bu.get_walrus_args
    def _gwa(*a, **kw):
        return _orig_gwa(*a, **kw) + ["--policy=2", "--global-dma-ordering-optimization=1", "--enable-hwdge-trigger-engine-scheduling"]
    _bu.get_walrus_args = _gwa

V = 50000
NCORES = 8
VC = V // NCORES          # 6250 owned vertices per core
P = 128
NVT = (VC + P - 1) // P   # 49 vertex tiles per core
VCP = NVT * P             # 6272 padded
MAXNV = 8                 # tiles per chunk (PSUM: 8*128 fp32 = 2 banks)

CHUNKS = [4, 6, 8, 8, 8, 8, 6, 1]
assert sum(CHUNKS) == NVT
NCH = len(CHUNKS)
# load groups: chunks loaded together (each group also carries its aux slice)
LGROUPS = [[0], [1], [2], [3, 4], [5, 6], [7]]

BF = mybir.dt.float16
BF_NP = np.float16

LAST_RESULTS = None       # BassKernelResults of the most recent run (for test.py)


def build(nc: bass.Bass, NPC):
    """NPC[c] = slot count (max degree + 1 fold slot) for chunk c."""
    dt = mybir.dt
    vstart = [0]
    for nv in CHUNKS:
        vstart.append(vstart[-1] + nv)

    # ---- packed input blob layout (fp16 cols per partition row) ----
    # group 0 tables+aux | wr | group 1 tables+aux | group 2 ... |
    W_COLS = 512
    o_xp = [0] * NCH
    o_aux = [0] * NCH
    off = 0
    group_span = []
    for gi, chs in enumerate(LGROUPS):
        glo = off
        for c in chs:
            o_xp[c] = off
            off += CHUNKS[c] * 3 * NPC[c]
        for c in chs:
            o_aux[c] = off
            off += CHUNKS[c] * 9
        group_span.append((glo, off))
        if gi == 0:
            o_wr = off
            off += W_COLS
    TOT = off

    blob = nc.dram_tensor("blobf41", [P, TOT], BF, kind="ExternalInput")
    out = nc.dram_tensor("out", [P, NVT * P], dt.float16, kind="ExternalOutput")

    with tile.TileContext(nc) as tc:
        with (
            tc.tile_pool(name="c", bufs=1) as cpool,
            tc.tile_pool(name="w", bufs=8) as wpool,
            tc.tile_pool(name="pt", bufs=2, space="PSUM") as pst,
            tc.tile_pool(name="pg", bufs=3, space="PSUM") as psg,
        ):
            outsb = cpool.tile([P, NVT * P], dt.float16)
            ident = cpool.tile([P, P], BF)
            with tc.high_priority():
                make_identity(nc, ident[:])
            xsb = cpool.tile([P, TOT], BF, tag="xsb")
            w_t = xsb[:, o_wr : o_wr + W_COLS]
            # loads: table groups stream on Sync; the W panel loads in
            # parallel from GpSimd so it never delays the chunk tables
            nc.sync.dma_start(
                out=xsb[:, group_span[0][0] : group_span[0][1]],
                in_=blob[:, group_span[0][0] : group_span[0][1]],
            )
            nc.gpsimd.dma_start(
                out=xsb[:, o_wr : o_wr + W_COLS], in_=blob[:, o_wr : o_wr + W_COLS]
            )
            for lo, hi in group_span[1:]:
                nc.sync.dma_start(out=xsb[:, lo:hi], in_=blob[:, lo:hi])

            # persistent u tiles (4-deep rotation); bias slot 9 = 1, 10:16 = 0
            u_bufs = []
            for s in range(4):
                ub = cpool.tile([P, MAXNV * 16], BF, tag=f"u{s}")
                nc.gpsimd.memset(ub[:], 0.0)
                nc.gpsimd.memset(
                    ub[:].rearrange("p (v s) -> p v s", s=16)[:, :, 9:10], 1.0
                )
                u_bufs.append(ub)

            state = [None] * NCH  # per-chunk (u, uT, pg)

            def stage_reduce_mult(c):
                nv = CHUNKS[c]
                npc = NPC[c]
                xv = xsb[:, o_xp[c] : o_xp[c] + nv * 3 * npc].rearrange(
                    "p (v c n) -> p v c n", v=nv, c=3, n=npc
                )
                t = wpool.tile([P, MAXNV * 3], BF, tag="t")
                with nc.allow_low_precision(reason="fp16 neighbor sums"):
                    nc.vector.tensor_reduce(
                        out=t[:, : nv * 3], in_=xv,
                        axis=mybir.AxisListType.X,
                        op=mybir.AluOpType.add,
                    )
                # u[p, v, k*3+j] = t[p,v,j]*lrf9[p,v,k*3+j] broadcast mul (Pool)
                u = u_bufs[c % 4]
                u9 = u[:, : nv * 16].rearrange("p (v s) -> p v s", s=16)[
                    :, :, 0:9
                ].rearrange("p v (k j) -> p v k j", k=3, j=3)
                aux9 = xsb[:, o_aux[c] : o_aux[c] + nv * 9].rearrange(
                    "p (v f) -> p v f", f=9
                )
                t4 = t[:, : nv * 3].rearrange("p (v c) -> p v c", c=3).unsqueeze(2)
                nc.gpsimd.tensor_tensor(
                    out=u9,
                    in0=t4.to_broadcast([P, nv, 3, 3]),
                    in1=aux9.rearrange("p v (k j) -> p v k j", k=3, j=3),
                    op=mybir.AluOpType.mult,
                )
                state[c] = [u, None, None]

            def stage_tu(c):
                nv = CHUNKS[c]
                cw = nv * 16
                u = state[c][0]
                pt = pst.tile([P, P], BF, tag="pt")
                nc.tensor.transpose(
                    out=pt[:cw, :], in_=u[:, :cw], identity=ident[:]
                )
                uT = wpool.tile([P, P], BF, tag="uT")
                nc.vector.tensor_copy(out=uT[:cw, :], in_=pt[:cw, :])
                state[c][1] = uT

            def stage_gemm(c):
                nv = CHUNKS[c]
                uT = state[c][1]
                pg = psg.tile([P, MAXNV * P], dt.float32, tag="pg")
                g = 0
                while g < nv:
                    ng = min(4, nv - g)
                    rb = 16 * g
                    nc.tensor.matmul(
                        out=pg[:, g * P : (g + ng) * P],
                        lhsT=uT[rb : rb + 16 * ng, :],
                        rhs=w_t[rb : rb + 16 * ng, : ng * P],
                        start=True,
                        stop=True,
                    )
                    g += ng
                state[c][2] = pg

            # every chunk flushes its own region right after its drain
            flush_of = {c: (c, c + 1) for c in range(NCH)}

            def stage_drain_store(c):
                nv = CHUNKS[c]
                ow = nv * P
                olo = vstart[c] * P
                pg = state[c][2]
                nc.scalar.copy(out=outsb[:, olo : olo + ow], in_=pg[:, :ow])
                if c in flush_of:
                    ca, cb = flush_of[c]
                    lo = vstart[ca] * P
                    hi = vstart[cb] * P
                    eng = nc.sync if c < NCH - 1 else nc.scalar
                    eng.dma_start(out=out[:, lo:hi], in_=outsb[:, lo:hi])

            # software-pipelined emission: reduce/mult 2 chunks ahead of the
            # GEMM, transpose/uT 1 ahead, drain right after its GEMM
            for i in range(NCH + 3):
                if i < NCH:
                    stage_reduce_mult(i)
                if 0 <= i - 1 < NCH:
                    stage_tu(i - 1)
                if 0 <= i - 2 < NCH:
                    stage_gemm(i - 2)
                if 0 <= i - 3 < NCH:
                    stage_drain_store(i - 3)
    return nc


def _host_prep(verts, edges, lrf, W, b):
    vb = np.asarray(verts, dtype=np.float32)
    e = np.asarray(edges).astype(np.int64)
    src = np.concatenate([e[:, 0], e[:, 1]]).astype(np.int64)
    dst = np.concatenate([e[:, 1], e[:, 0]]).astype(np.int64)

    deg = np.bincount(src, minlength=V).astype(np.int64)
    maxN = int(deg.max())

    # per-core remap: sort by degree ascending -> low-degree chunks get
    # narrow neighbor tables
    degc = deg.reshape(NCORES, VC)
    newpos = np.empty((NCORES, VC), np.int64)
    order_c = np.empty((NCORES, VC), np.int64)
    for cc in range(NCORES):
        oc = np.argsort(degc[cc], kind="stable")
        order_c[cc] = oc
        newpos[cc, oc] = np.arange(VC)

    vstart = [0]
    for nv in CHUNKS:
        vstart.append(vstart[-1] + nv)
    deg_sorted = np.sort(degc, axis=1)          # per core, ascending
    NPC = []
    for c in range(NCH):
        hi = min(vstart[c + 1] * P, VC)
        cap = int(deg_sorted[:, :hi].max())     # max over cores for SPMD
        NPC.append(cap + 1)

    order = np.argsort(src, kind="stable")
    src_s = src[order]
    dst_s = dst[order]
    starts = np.zeros(V + 1, np.int64)
    np.cumsum(deg, out=starts[1:])
    slot = np.arange(src_s.size, dtype=np.int64) - starts[src_s]

    c_a = src_s // VC
    il_new = newpos[c_a, src_s - c_a * VC]
    p_a = il_new % P
    v_a = il_new // P
    vals = vb[dst_s].astype(BF_NP)

    chunk_of_tile = np.zeros(NVT, np.int64)
    for c in range(NCH):
        chunk_of_tile[vstart[c] : vstart[c + 1]] = c
    ch_a = chunk_of_tile[v_a]

    Xp = [np.zeros((NCORES, P, CHUNKS[c], 3, NPC[c]), BF_NP) for c in range(NCH)]
    for c in range(NCH):
        m = ch_a == c
        Xp[c][c_a[m], p_a[m], v_a[m] - vstart[c], :, slot[m]] = vals[m]

    # fold slot: -deg*verts for the owned vertex goes in the last slot
    dv = (-deg[:, None].astype(np.float32)) * vb
    dv_pad = np.zeros((NCORES, VCP, 3), np.float32)
    for cc in range(NCORES):
        dv_pad[cc, :VC] = dv.reshape(NCORES, VC, 3)[cc][order_c[cc]]
    dv_t = dv_pad.reshape(NCORES, NVT, P, 3).transpose(0, 2, 1, 3)  # [NC,P,NVT,3]
    for c in range(NCH):
        Xp[c][:, :, :, :, NPC[c] - 1] = dv_t[
            :, :, vstart[c] : vstart[c + 1], :
        ].astype(BF_NP)

    # aux per vertex: lrf(9), remapped -> [NC, P, NVT, 9]
    aux_flat = np.zeros((NCORES, VCP, 9), np.float32)
    # k-major flattening: slot s = k*3+j holds lrf[:, j, k]
    lrf9 = np.ascontiguousarray(
        np.asarray(lrf, np.float32).reshape(NCORES, VC, 3, 3).transpose(0, 1, 3, 2)
    ).reshape(NCORES, VC, 9)
    for cc in range(NCORES):
        aux_flat[cc, :VC] = lrf9[cc][order_c[cc]]
    auxh = aux_flat.reshape(NCORES, NVT, P, 9).transpose(0, 2, 1, 3).astype(BF_NP)

    Wf = np.asarray(W, np.float32)
    W16 = np.zeros((16, P), np.float32)
    for s in range(9):
        W16[s, :] = Wf[:, s // 3]   # k-major: slot s = k*3+j -> k = s//3
    W16[9, :] = maxN * np.asarray(b, np.float32)
    half = np.zeros((64, 512), np.float32)
    for q in range(4):
        half[16 * q : 16 * q + 16, 128 * q : 128 * q + 128] = W16
    Wr = np.ascontiguousarray(np.vstack([half, half])).astype(BF_NP)

    in_maps = []
    for cc in range(NCORES):
        parts = []
        for gi, chs in enumerate(LGROUPS):
            for c in chs:
                parts.append(np.ascontiguousarray(Xp[c][cc].reshape(P, -1)))
            for c in chs:
                parts.append(
                    np.ascontiguousarray(
                        auxh[cc, :, vstart[c] : vstart[c + 1]].reshape(P, -1)
                    )
                )
            if gi == 0:
                parts.append(Wr)
        in_maps.append({"blobf41": np.ascontiguousarray(np.concatenate(parts, axis=1))})
    return in_maps, NPC, order_c


def kernel(verts, edges, lrf, W, b):
    global LAST_RESULTS
    in_maps, NPC, order_c = _host_prep(verts, edges, lrf, W, b)

    nc = bacc.Bacc()
    build(nc, NPC)
    nc.finalize()

    trace = os.environ.get("KBENCH_TRACE") == "1"
    res = run_bass_kernel_spmd(
        nc, in_maps, core_ids=list(range(NCORES)), trace=trace
    )
    LAST_RESULTS = res

    full = np.empty((V, 128), np.float32)
    for c in range(NCORES):
        o = (
            res.results[c]["out"].astype(np.float32)
            .reshape(P, NVT, P).transpose(1, 0, 2).reshape(VCP, P)[:VC]
        )
        blk = full[c * VC : (c + 1) * VC]
        blk[order_c[c]] = o
    return full
